# revision 41
# baseline (speedup 1.0000x reference)
"""Trainium2 Bass kernel for AttentionReadoutAtom (global-softmax segment reduce).

Math:  scores = x @ w + b ; attn = softmax(scores over all N) ;
       out[s] = sum_{i: label_i = s} attn_i * x_i          -> [50000, 128]

Softmax is shift/scale invariant: exp(score) without max-subtraction is safe
here (scores ~ N(0,1)), and the bias b cancels between numerator and
denominator.  Using xw = x * w (host-side sharding layout prep):

    out[s, d] = sum_{i in s} e_i * xw_i[d] / (w[d] * Z),   Z = sum_i e_i

Sharding (host, inside kernel()):
  * Sort rows by segment label; greedily pack whole segments into blocks of
    512 rows (4 tiles of 128 rows) covering <= 48 distinct segments each;
    pad each block to 512 rows with zero rows.  Every segment lives in
    exactly one block -> no cross-core combination of outputs is needed;
    the only global quantity is the softmax denominator Z, reduced on the
    host from the per-row e values (the hint's denominator all-reduce).
    The 48-seg geometry keeps the one-hot matrix small: [128, 4*48]
    per block instead of [128, 8*128] — 5x fewer GPSIMD-written bytes.
  * Blocks are dealt contiguously to 8 cores, padded to equal count B
    (a multiple of 4); chunks of 8 blocks share one contiguous
    8KB-per-partition xw DMA ([P, B*512] dram layout).
  * idx[p, b*4+t] = (b%4)*192 + t*48 + rel_label, or -1 for pad rows,
    int16 — drives the device-side one-hot build, pre-offset so four
    blocks share a single local_scatter call.

Device per chunk ch (8 blocks; front half emitted one chunk ahead of the
back half so the ScalarE exp is never queued behind PE-gated evicts):
  * scores: three bf16 halving levels (DVE tensor_tensor, 2x mode) then
    one 1x tensor_reduce over the last 16 -> sc [P, 32] f32 per chunk.
  * e = exp(score) — ONE ScalarE ACTIVATE [P, 32] -> persistent zbuf
    (bf16); zbuf doubles as the Z output, DMA'd out in two halves.
  * Me[p, u*192 + t*48 + s] = e one-hot — ONE GPSIMD local_scatter per
    4-block group (dst zeroed by the op; -1 pad indices skipped).  This
    keeps the former DVE/ScalarE per-tile one-hot bottleneck on the
    otherwise idle GPSIMD engine at ~0.9us per 2048 rows.
  * psum[s, 4*128] += Me_t^T @ xw_t — TensorE, 16 matmuls per group
    accumulating into one shared PSUM tile.
  * ONE ScalarE Copy evicts the group's psum into a persistent obuf;
    obuf flushes to DRAM in eighths, issued from the ScalarE queue one
    chunk late (wait-free, sync queue never blocked).

Host epilogue: scatter per-block rows to the full [50000, 128] output and
apply the scalar normalization out / (w[d] * Z).

Measured on 8 trn2 NeuronCores (NTFF profile, max across cores): ~87-97 us
(mean across cores ~85 us, run-to-run max-core variance +-8 us), vs the
326 us one-hot-on-DVE baseline.  Remaining limits: 16.3 MB/core xw input
at ~355 GB/s DMA (~48 us), DVE score tree ~50 us, ~10 us preamble+first
DMA latency, ~8 us tail (final flush + postamble).
"""

import numpy as np
import ml_dtypes

# ---------------------------------------------------------------- constants
N = 500000
D = 128
NUM_SEGMENTS = 50000
N_CORES = 8
P = 128
TPB = 4                   # row tiles per block
ROWS_PER_BLOCK = TPB * P  # 512
SEGS = 48                 # max distinct segments per block
MAX_SEGS_PER_BLOCK = SEGS
CHUNK_BLOCKS = 8          # blocks per chunk
GB = 4                    # blocks per scatter/psum/evict group

_COMPILED = {}


# ---------------------------------------------------------------- device code
def _build_kernel(B):
    import concourse.bacc as bacc
    import concourse.mybir as mybir
    from concourse.tile import TileContext

    f32 = mybir.dt.float32
    bf16 = mybir.dt.bfloat16
    i16 = mybir.dt.int16
    Alu = mybir.AluOpType
    Act = mybir.ActivationFunctionType
    Ax = mybir.AxisListType

    W = TPB * P                      # xw columns per block (512)
    MW = TPB * SEGS                  # one-hot columns per block (192)
    NCHUNK = (B + CHUNK_BLOCKS - 1) // CHUNK_BLOCKS
    CC = CHUNK_BLOCKS * TPB          # score/e columns per full chunk

    nc = bacc.Bacc("TRN2", target_bir_lowering=False, debug=False,
                   num_devices=N_CORES)

    xw_d = nc.dram_tensor("xw", [P, B * W], bf16, kind="ExternalInput")
    idx_d = nc.dram_tensor("idx", [P, B * TPB], i16, kind="ExternalInput")
    out_d = nc.dram_tensor("out", [SEGS, B * P], bf16, kind="ExternalOutput")
    z_d = nc.dram_tensor("zpart", [P, B * TPB], bf16, kind="ExternalOutput")

    with TileContext(nc) as tc:
        with tc.tile_pool(name="const", bufs=1) as cpool, \
             tc.tile_pool(name="xwp", bufs=6) as xwp, \
             tc.tile_pool(name="hp", bufs=12) as hp, \
             tc.tile_pool(name="scp", bufs=6) as scp, \
             tc.tile_pool(name="mep", bufs=24) as mep, \
             tc.tile_pool(name="psum", bufs=8, space="PSUM") as psp:

            idx_t = cpool.tile([P, B * TPB], i16)
            zbuf = cpool.tile([P, B * TPB], bf16)
            # dummy no-op scatter (all indices -1): forces the gpsimd
            # local_scatter library load to happen during the preamble,
            # not on the critical path of the first real scatter
            dum_i = cpool.tile([P, 2], i16)
            dum_d = cpool.tile([P, 2], bf16)
            dum_o = cpool.tile([P, 2], bf16)
            nc.vector.memset(dum_i[:], -1)
            nc.vector.memset(dum_d[:], 0)
            nc.gpsimd.local_scatter(
                out_ap=dum_o[:], data_ap=dum_d[:], idxs_ap=dum_i[:],
                channels=P, num_elems=2, num_idxs=2)
            obuf = cpool.tile([P, B * P], bf16)  # only rows :SEGS used
            flushed = 0
            z_flushed = 0

            def issue_flush(pend, pend_z=None):
                # issued from the ScalarE queue one chunk late: the evicts
                # (resp. exps) covering the range have already executed on
                # this very queue, so the DMA's wait is free; the sync
                # queue (xw prefetch) is never blocked, and the GPSIMD
                # pacer queue is not burdened with the ~1.4us descriptor
                # generation a gpsimd-issued DMA costs.
                if pend is not None:
                    f0, f1 = pend
                    nc.scalar.dma_start(
                        out_d.ap()[:, f0 * P:f1 * P],
                        obuf[0:SEGS, f0 * P:f1 * P])
                if pend_z is not None:
                    z0, z1 = pend_z
                    nc.scalar.dma_start(
                        z_d.ap()[:, z0 * TPB:z1 * TPB],
                        zbuf[:, z0 * TPB:z1 * TPB])

            def front_half(ch):
                """DMA + score pipeline (DVE/ScalarE) for chunk ch.
                Emitted one chunk AHEAD of the chunk's scatters/matmuls
                so that on the ScalarE queue the exp for chunk ch sits
                BEFORE the evicts of chunk ch-1 (which wait on the PE) —
                without this the chunk chain scatters -> matmuls ->
                evicts -> next exp is fully serial.
                """
                b0 = ch * CHUNK_BLOCKS
                nb = min(CHUNK_BLOCKS, B - b0)
                xw_c = xwp.tile([P, CHUNK_BLOCKS * W], bf16, tag="xw")
                h_c = hp.tile([P, CC * 64], bf16, tag="h")
                h2_c = hp.tile([P, CC * 32], bf16, tag="h2")
                h3_c = hp.tile([P, CC * 16], bf16, tag="h3")
                sc_c = scp.tile([P, CC], f32, tag="sc")
                x4 = xw_c[:].rearrange("p (t d) -> p t d", d=P)
                h4 = h_c[:].rearrange("p (t j) -> p t j", j=64)
                h24 = h2_c[:].rearrange("p (t j) -> p t j", j=32)
                h34 = h3_c[:].rearrange("p (t j) -> p t j", j=16)
                # chunk 0 is processed in group-sized pieces so the first
                # scatter group starts sooner; later chunks use one piece
                PZ = GB if ch == 0 else CHUNK_BLOCKS
                for hb in range(0, nb, PZ):
                    nh = min(PZ, nb - hb)
                    nc.sync.dma_start(
                        xw_c[:, hb * W:(hb + nh) * W],
                        xw_d.ap()[:, (b0 + hb) * W:(b0 + hb + nh) * W])
                    if ch == 0 and hb == 0:
                        # idx is first needed by the first scatter, well
                        # after the first xw piece — don't delay that piece
                        nc.sync.dma_start(idx_t[:], idx_d.ap()[:, :])
                    # three halving levels of the score row-sums in
                    # 2x-mode ops, then one 1x tensor_reduce and one exp
                    t0, t1 = hb * TPB, (hb + nh) * TPB
                    nc.vector.tensor_tensor(
                        out=h4[:, t0:t1, :],
                        in0=x4[:, t0:t1, 0:64], in1=x4[:, t0:t1, 64:128],
                        op=Alu.add)
                    nc.vector.tensor_tensor(
                        out=h24[:, t0:t1, :],
                        in0=h4[:, t0:t1, 0:32], in1=h4[:, t0:t1, 32:64],
                        op=Alu.add)
                    nc.vector.tensor_tensor(
                        out=h34[:, t0:t1, :],
                        in0=h24[:, t0:t1, 0:16], in1=h24[:, t0:t1, 16:32],
                        op=Alu.add)
                    nc.vector.tensor_reduce(
                        out=sc_c[:, t0:t1], in_=h34[:, t0:t1, :],
                        axis=Ax.X, op=Alu.add)
                    nc.scalar.activation(
                        out=zbuf[:, b0 * TPB + t0:b0 * TPB + t1],
                        in_=sc_c[:, t0:t1], func=Act.Exp)
                return ch, b0, nb, xw_c

            def back_half(state):
                ch, b0, nb, xw_c = state
                # four consecutive blocks share one local_scatter (their
                # one-hot columns are packed side by side via the baked
                # u*MW index offset), one PSUM tile, and one evict
                for bi in range(0, nb, GB):
                    ng = min(GB, nb - bi)
                    b = b0 + bi
                    me_t = mep.tile([P, GB * MW], bf16, tag="me")
                    nc.gpsimd.local_scatter(
                        out_ap=me_t[:, :ng * MW],
                        data_ap=zbuf[:, b * TPB:(b + ng) * TPB],
                        idxs_ap=idx_t[:, b * TPB:(b + ng) * TPB],
                        channels=P, num_elems=ng * MW, num_idxs=ng * TPB)
                    ps = psp.tile([SEGS, GB * P], f32, tag="acc")
                    for u in range(ng):
                        for t in range(TPB):
                            nc.tensor.matmul(
                                ps[:, u * P:(u + 1) * P],
                                lhsT=me_t[:, u * MW + t * SEGS:
                                          u * MW + (t + 1) * SEGS],
                                rhs=xw_c[:, ((bi + u) * TPB + t) * P:
                                         ((bi + u) * TPB + t + 1) * P],
                                start=(t == 0), stop=(t == TPB - 1))
                    nc.scalar.activation(
                        out=obuf[0:SEGS, b * P:(b + ng) * P],
                        in_=ps[:, :ng * P], func=Act.Copy)

            pend = pend_z = None
            prev = front_half(0)
            for ch in range(1, NCHUNK):
                nxt = front_half(ch)
                back_half(prev)
                prev = nxt

                done = ch * CHUNK_BLOCKS    # blocks fully evicted so far
                if done - flushed >= (B + 7) // 8:
                    pend = (flushed, done)
                    flushed = done
                if done * 2 >= B and z_flushed == 0:
                    pend_z = (z_flushed, done)
                    z_flushed = done
                issue_flush(pend, pend_z)
                pend = pend_z = None
            back_half(prev)
            issue_flush((flushed, B), (z_flushed, B))

    nc.compile()
    return nc


# ---------------------------------------------------------------- host side
def _pack_blocks(counts):
    blocks = []
    s, nseg = 0, len(counts)
    while s < nseg:
        rows, s0 = 0, s
        while s < nseg and s - s0 < MAX_SEGS_PER_BLOCK:
            c = counts[s]
            if rows + c > ROWS_PER_BLOCK:
                break
            rows += int(c)
            s += 1
        assert s > s0, f"segment {s0} with {counts[s0]} rows exceeds a block"
        blocks.append((s0, s, rows))
    return blocks


def _numpy_fallback(x, labels, w, b):
    scores = x.astype(np.float64) @ w.astype(np.float64) + float(b)
    scores -= scores.max()
    e = np.exp(scores)
    a = e / e.sum()
    out = np.zeros((NUM_SEGMENTS, x.shape[1]), np.float64)
    np.add.at(out, labels, x * a[:, None])
    return out.astype(np.float32)


def kernel(x, monomer_labels_i, attn_w, attn_b):
    from concourse import bass_utils

    x = np.ascontiguousarray(np.asarray(x, dtype=np.float32))
    labels = np.asarray(monomer_labels_i).astype(np.int64)
    w = np.asarray(attn_w, dtype=np.float32)
    b = np.float32(np.asarray(attn_b))

    if np.abs(w).min() < 1e-30 or np.bincount(
            labels, minlength=NUM_SEGMENTS).max() > ROWS_PER_BLOCK:
        return _numpy_fallback(x, labels, w, b)

    order = np.argsort(labels, kind="stable")
    labels_s = labels[order]
    counts = np.bincount(labels, minlength=NUM_SEGMENTS)
    blocks = _pack_blocks(counts)
    nblocks = len(blocks)
    B = (nblocks + N_CORES - 1) // N_CORES
    B = (B + GB - 1) // GB * GB       # whole scatter groups
    NT = B * TPB
    seg_row_start = np.zeros(NUM_SEGMENTS + 1, np.int64)
    np.cumsum(counts, out=seg_row_start[1:])

    xw = x[order] * w[None, :]
    xw_hi = xw.astype(ml_dtypes.bfloat16)

    # per-tile one-hot column index: t*SEGS + rel_label (pad rows: -1)
    tile_base = (np.arange(ROWS_PER_BLOCK) // P).astype(np.int16) * SEGS

    in_maps = []
    meta = []
    for c in range(N_CORES):
        xw_dev = np.zeros((B, P, TPB, P), ml_dtypes.bfloat16)
        idx_dev = np.full((B, TPB, P), -1, np.int16)
        meta_c = []
        for bi in range(B):
            gi = c * B + bi
            if gi >= nblocks:
                meta_c.append(None)
                continue
            s0, s1, rows = blocks[gi]
            r0 = seg_row_start[s0]

            full = np.zeros((ROWS_PER_BLOCK, D), ml_dtypes.bfloat16)
            full[:rows] = xw_hi[r0:r0 + rows]
            xw_dev[bi] = full.reshape(TPB, P, D).transpose(1, 0, 2)

            fi = np.full(ROWS_PER_BLOCK, -1, np.int16)
            fi[:rows] = (labels_s[r0:r0 + rows] - s0).astype(np.int16) + \
                tile_base[:rows] + np.int16((bi % GB) * TPB * SEGS)
            idx_dev[bi] = fi.reshape(TPB, P)
            meta_c.append((int(s0), int(s1)))
        meta.append(meta_c)
        # idx layout on device: [P, B*TPB], column b*TPB+t
        in_maps.append({"xw": np.ascontiguousarray(
                            xw_dev.reshape(B, P, TPB * P)
                            .transpose(1, 0, 2).reshape(P, B * TPB * P)),
                        "idx": np.ascontiguousarray(
                            idx_dev.reshape(NT, P).T)})

    if B not in _COMPILED:
        _COMPILED[B] = _build_kernel(B)
    nc = _COMPILED[B]

    res = bass_utils.run_bass_kernel_spmd(nc, in_maps,
                                          core_ids=list(range(N_CORES)))

    # ---- gather / unshard
    out = np.zeros((NUM_SEGMENTS, D), np.float32)
    Z = 0.0
    for c in range(N_CORES):
        r = res.results[c]
        Z += float(r["zpart"].astype(np.float64).sum())
        out_dev = r["out"].reshape(SEGS, B, P).transpose(1, 0, 2)
        for bi in range(B):
            m = meta[c][bi]
            if m is None:
                continue
            s0, s1 = m
            out[s0:s1] = out_dev[bi, :s1 - s0, :].astype(np.float32)
    # pad rows have xw == 0 -> score 0 -> e = exp(0) = 1 each
    n_pad_rows = N_CORES * B * ROWS_PER_BLOCK - N
    Z -= float(n_pad_rows)
    out /= (w[None, :] * np.float32(Z))
    return out.astype(np.float32)


if __name__ == "__main__":
    from ref_io import get
    inputs, expected = get()
    out = kernel(**inputs)
    err = np.abs(out - expected)
    print("absmax err:", err.max(), "scale-rel:",
          err.max() / np.abs(expected).max())


# revision 42
# speedup vs baseline: 1.0522x; 1.0522x over previous
"""Trainium2 Bass kernel for AttentionReadoutAtom (global-softmax segment reduce).

Math:  scores = x @ w + b ; attn = softmax(scores over all N) ;
       out[s] = sum_{i: label_i = s} attn_i * x_i          -> [50000, 128]

Softmax is shift/scale invariant: exp(score) without max-subtraction is safe
here (scores ~ N(0,1)), and the bias b cancels between numerator and
denominator.  Using xw = x * w (host-side sharding layout prep):

    out[s, d] = sum_{i in s} e_i * xw_i[d] / (w[d] * Z),   Z = sum_i e_i

Sharding (host, inside kernel()):
  * Sort rows by segment label; greedily pack whole segments into blocks of
    512 rows (4 tiles of 128 rows) covering <= 48 distinct segments each;
    pad each block to 512 rows with zero rows.  Every segment lives in
    exactly one block -> no cross-core combination of outputs is needed;
    the only global quantity is the softmax denominator Z, reduced on the
    host from the per-row e values (the hint's denominator all-reduce).
    The 48-seg geometry keeps the one-hot matrix small: [128, 4*48]
    per block instead of [128, 8*128] — 5x fewer GPSIMD-written bytes.
  * Blocks are dealt contiguously to 8 cores, padded to equal count B
    (a multiple of 4); chunks of 8 blocks share one contiguous
    8KB-per-partition xw DMA ([P, B*512] dram layout).
  * idx[p, b*4+t] = (b%4)*192 + t*48 + rel_label, or -1 for pad rows,
    int16 — drives the device-side one-hot build, pre-offset so four
    blocks share a single local_scatter call.

Device per chunk ch (8 blocks; front half emitted one chunk ahead of the
back half so the ScalarE exp is never queued behind PE-gated evicts):
  * scores: three bf16 halving levels (DVE tensor_tensor, 2x mode) then
    one 1x tensor_reduce over the last 16 -> sc [P, 32] f32 per chunk.
  * e = exp(score) — ONE ScalarE ACTIVATE [P, 32] -> persistent zbuf
    (bf16); zbuf doubles as the Z output, DMA'd out in two halves.
  * Me[p, u*192 + t*48 + s] = e one-hot — ONE GPSIMD local_scatter per
    4-block group (dst zeroed by the op; -1 pad indices skipped).  This
    keeps the former DVE/ScalarE per-tile one-hot bottleneck on the
    otherwise idle GPSIMD engine at ~0.9us per 2048 rows.
  * psum[s, 4*128] += Me_t^T @ xw_t — TensorE, 16 matmuls per group
    accumulating into one shared PSUM tile.
  * ONE ScalarE Copy evicts the group's psum into a persistent obuf;
    obuf flushes to DRAM in eighths, issued from the ScalarE queue one
    chunk late (wait-free, sync queue never blocked).

Host epilogue: scatter per-block rows to the full [50000, 128] output and
apply the scalar normalization out / (w[d] * Z).

Measured on 8 trn2 NeuronCores (NTFF profile, max across cores): ~87-97 us
(mean across cores ~85 us, run-to-run max-core variance +-8 us), vs the
326 us one-hot-on-DVE baseline.  Remaining limits: 16.3 MB/core xw input
at ~355 GB/s DMA (~48 us), DVE score tree ~50 us, ~10 us preamble+first
DMA latency, ~8 us tail (final flush + postamble).
"""

import numpy as np
import ml_dtypes

# ---------------------------------------------------------------- constants
N = 500000
D = 128
NUM_SEGMENTS = 50000
N_CORES = 8
P = 128
TPB = 4                   # row tiles per block
ROWS_PER_BLOCK = TPB * P  # 512
SEGS = 48                 # max distinct segments per block
MAX_SEGS_PER_BLOCK = SEGS
CHUNK_BLOCKS = 8          # blocks per chunk
GB = 4                    # blocks per scatter/psum/evict group

_COMPILED = {}


# ---------------------------------------------------------------- device code
def _build_kernel(B):
    import concourse.bacc as bacc
    import concourse.mybir as mybir
    from concourse.tile import TileContext

    f32 = mybir.dt.float32
    bf16 = mybir.dt.bfloat16
    i16 = mybir.dt.int16
    Alu = mybir.AluOpType
    Act = mybir.ActivationFunctionType
    Ax = mybir.AxisListType

    W = TPB * P                      # xw columns per block (512)
    MW = TPB * SEGS                  # one-hot columns per block (192)
    NCHUNK = (B + CHUNK_BLOCKS - 1) // CHUNK_BLOCKS
    CC = CHUNK_BLOCKS * TPB          # score/e columns per full chunk

    nc = bacc.Bacc("TRN2", target_bir_lowering=False, debug=False,
                   num_devices=N_CORES)

    xw_d = nc.dram_tensor("xw", [P, B * W], bf16, kind="ExternalInput")
    idx_d = nc.dram_tensor("idx", [P, B * TPB], i16, kind="ExternalInput")
    out_d = nc.dram_tensor("out", [SEGS, B * P], bf16, kind="ExternalOutput")
    z_d = nc.dram_tensor("zpart", [P, B * TPB], bf16, kind="ExternalOutput")

    with TileContext(nc) as tc:
        with tc.tile_pool(name="const", bufs=1) as cpool, \
             tc.tile_pool(name="xwp", bufs=6) as xwp, \
             tc.tile_pool(name="hp", bufs=12) as hp, \
             tc.tile_pool(name="scp", bufs=6) as scp, \
             tc.tile_pool(name="mep", bufs=24) as mep, \
             tc.tile_pool(name="psum", bufs=8, space="PSUM") as psp:

            idx_t = cpool.tile([P, B * TPB], i16)
            zbuf = cpool.tile([P, B * TPB], bf16)
            # dummy no-op scatter (all indices -1): forces the gpsimd
            # local_scatter library load to happen during the preamble,
            # not on the critical path of the first real scatter
            dum_i = cpool.tile([P, 2], i16)
            dum_d = cpool.tile([P, 2], bf16)
            dum_o = cpool.tile([P, 2], bf16)
            nc.vector.memset(dum_i[:], -1)
            nc.vector.memset(dum_d[:], 0)
            nc.gpsimd.local_scatter(
                out_ap=dum_o[:], data_ap=dum_d[:], idxs_ap=dum_i[:],
                channels=P, num_elems=2, num_idxs=2)
            obuf = cpool.tile([P, B * P], bf16)  # only rows :SEGS used
            flushed = 0
            z_flushed = 0

            def issue_flush(pend, pend_z=None):
                # issued from the ScalarE queue one chunk late: the evicts
                # (resp. exps) covering the range have already executed on
                # this very queue, so the DMA's wait is free; the sync
                # queue (xw prefetch) is never blocked, and the GPSIMD
                # pacer queue is not burdened with the ~1.4us descriptor
                # generation a gpsimd-issued DMA costs.
                if pend is not None:
                    f0, f1 = pend
                    nc.scalar.dma_start(
                        out_d.ap()[:, f0 * P:f1 * P],
                        obuf[0:SEGS, f0 * P:f1 * P])
                if pend_z is not None:
                    z0, z1 = pend_z
                    nc.scalar.dma_start(
                        z_d.ap()[:, z0 * TPB:z1 * TPB],
                        zbuf[:, z0 * TPB:z1 * TPB])

            def front_a(ch):
                """xw DMA + first halving level for chunk ch.  Emitted one
                chunk ahead of front_b so that on the in-order DVE queue
                TT2(ch) arrives after TT1(ch+1), by which time TT1(ch)'s
                pipeline drain is long finished — its wait is free."""
                b0 = ch * CHUNK_BLOCKS
                nb = min(CHUNK_BLOCKS, B - b0)
                xw_c = xwp.tile([P, CHUNK_BLOCKS * W], bf16, tag="xw")
                h_c = hp.tile([P, CC * 64], bf16, tag="h")
                x4 = xw_c[:].rearrange("p (t d) -> p t d", d=P)
                h4 = h_c[:].rearrange("p (t j) -> p t j", j=64)
                # chunk 0 is processed in group-sized pieces so the first
                # scatter group starts sooner; later chunks use one piece
                PZ = GB if ch == 0 else CHUNK_BLOCKS
                for hb in range(0, nb, PZ):
                    nh = min(PZ, nb - hb)
                    nc.sync.dma_start(
                        xw_c[:, hb * W:(hb + nh) * W],
                        xw_d.ap()[:, (b0 + hb) * W:(b0 + hb + nh) * W])
                    if ch == 0 and hb == 0:
                        # idx is first needed by the first scatter, well
                        # after the first xw piece — don't delay that piece
                        nc.sync.dma_start(idx_t[:], idx_d.ap()[:, :])
                    t0, t1 = hb * TPB, (hb + nh) * TPB
                    nc.vector.tensor_tensor(
                        out=h4[:, t0:t1, :],
                        in0=x4[:, t0:t1, 0:64], in1=x4[:, t0:t1, 64:128],
                        op=Alu.add)
                return ch, b0, nb, xw_c, h_c

            def front_b(st):
                """Remaining halvings + reduce + exp for chunk ch."""
                ch, b0, nb, xw_c, h_c = st
                h2_c = hp.tile([P, CC * 32], bf16, tag="h2")
                h3_c = hp.tile([P, CC * 16], bf16, tag="h3")
                sc_c = scp.tile([P, CC], f32, tag="sc")
                h4 = h_c[:].rearrange("p (t j) -> p t j", j=64)
                h24 = h2_c[:].rearrange("p (t j) -> p t j", j=32)
                h34 = h3_c[:].rearrange("p (t j) -> p t j", j=16)
                nt = nb * TPB
                nc.vector.tensor_tensor(
                    out=h24[:, :nt, :],
                    in0=h4[:, :nt, 0:32], in1=h4[:, :nt, 32:64],
                    op=Alu.add)
                nc.vector.tensor_tensor(
                    out=h34[:, :nt, :],
                    in0=h24[:, :nt, 0:16], in1=h24[:, :nt, 16:32],
                    op=Alu.add)
                nc.vector.tensor_reduce(
                    out=sc_c[:, :nt], in_=h34[:, :nt, :],
                    axis=Ax.X, op=Alu.add)
                nc.scalar.activation(
                    out=zbuf[:, b0 * TPB:b0 * TPB + nt],
                    in_=sc_c[:, :nt], func=Act.Exp)
                return ch, b0, nb, xw_c

            def back_half(state):
                ch, b0, nb, xw_c = state
                # four consecutive blocks share one local_scatter (their
                # one-hot columns are packed side by side via the baked
                # u*MW index offset), one PSUM tile, and one evict
                for bi in range(0, nb, GB):
                    ng = min(GB, nb - bi)
                    b = b0 + bi
                    me_t = mep.tile([P, GB * MW], bf16, tag="me")
                    nc.gpsimd.local_scatter(
                        out_ap=me_t[:, :ng * MW],
                        data_ap=zbuf[:, b * TPB:(b + ng) * TPB],
                        idxs_ap=idx_t[:, b * TPB:(b + ng) * TPB],
                        channels=P, num_elems=ng * MW, num_idxs=ng * TPB)
                    ps = psp.tile([SEGS, GB * P], f32, tag="acc")
                    for u in range(ng):
                        for t in range(TPB):
                            nc.tensor.matmul(
                                ps[:, u * P:(u + 1) * P],
                                lhsT=me_t[:, u * MW + t * SEGS:
                                          u * MW + (t + 1) * SEGS],
                                rhs=xw_c[:, ((bi + u) * TPB + t) * P:
                                         ((bi + u) * TPB + t + 1) * P],
                                start=(t == 0), stop=(t == TPB - 1))
                    nc.scalar.activation(
                        out=obuf[0:SEGS, b * P:(b + ng) * P],
                        in_=ps[:, :ng * P], func=Act.Copy)

            pend = pend_z = None
            fa = front_a(0)
            prev = front_b(fa)
            fa = front_a(1) if NCHUNK > 1 else None
            for ch in range(1, NCHUNK):
                nxt_fa = front_a(ch + 1) if ch + 1 < NCHUNK else None
                nxt = front_b(fa)
                back_half(prev)
                prev, fa = nxt, nxt_fa

                done = ch * CHUNK_BLOCKS    # blocks fully evicted so far
                if done - flushed >= (B + 7) // 8:
                    pend = (flushed, done)
                    flushed = done
                if done * 2 >= B and z_flushed == 0:
                    pend_z = (z_flushed, done)
                    z_flushed = done
                issue_flush(pend, pend_z)
                pend = pend_z = None
            back_half(prev)
            issue_flush((flushed, B), (z_flushed, B))

    nc.compile()
    return nc


# ---------------------------------------------------------------- host side
def _pack_blocks(counts):
    blocks = []
    s, nseg = 0, len(counts)
    while s < nseg:
        rows, s0 = 0, s
        while s < nseg and s - s0 < MAX_SEGS_PER_BLOCK:
            c = counts[s]
            if rows + c > ROWS_PER_BLOCK:
                break
            rows += int(c)
            s += 1
        assert s > s0, f"segment {s0} with {counts[s0]} rows exceeds a block"
        blocks.append((s0, s, rows))
    return blocks


def _numpy_fallback(x, labels, w, b):
    scores = x.astype(np.float64) @ w.astype(np.float64) + float(b)
    scores -= scores.max()
    e = np.exp(scores)
    a = e / e.sum()
    out = np.zeros((NUM_SEGMENTS, x.shape[1]), np.float64)
    np.add.at(out, labels, x * a[:, None])
    return out.astype(np.float32)


def kernel(x, monomer_labels_i, attn_w, attn_b):
    from concourse import bass_utils

    x = np.ascontiguousarray(np.asarray(x, dtype=np.float32))
    labels = np.asarray(monomer_labels_i).astype(np.int64)
    w = np.asarray(attn_w, dtype=np.float32)
    b = np.float32(np.asarray(attn_b))

    if np.abs(w).min() < 1e-30 or np.bincount(
            labels, minlength=NUM_SEGMENTS).max() > ROWS_PER_BLOCK:
        return _numpy_fallback(x, labels, w, b)

    order = np.argsort(labels, kind="stable")
    labels_s = labels[order]
    counts = np.bincount(labels, minlength=NUM_SEGMENTS)
    blocks = _pack_blocks(counts)
    nblocks = len(blocks)
    B = (nblocks + N_CORES - 1) // N_CORES
    B = (B + GB - 1) // GB * GB       # whole scatter groups
    NT = B * TPB
    seg_row_start = np.zeros(NUM_SEGMENTS + 1, np.int64)
    np.cumsum(counts, out=seg_row_start[1:])

    xw = x[order] * w[None, :]
    xw_hi = xw.astype(ml_dtypes.bfloat16)

    # per-tile one-hot column index: t*SEGS + rel_label (pad rows: -1)
    tile_base = (np.arange(ROWS_PER_BLOCK) // P).astype(np.int16) * SEGS

    in_maps = []
    meta = []
    for c in range(N_CORES):
        xw_dev = np.zeros((B, P, TPB, P), ml_dtypes.bfloat16)
        idx_dev = np.full((B, TPB, P), -1, np.int16)
        meta_c = []
        for bi in range(B):
            gi = c * B + bi
            if gi >= nblocks:
                meta_c.append(None)
                continue
            s0, s1, rows = blocks[gi]
            r0 = seg_row_start[s0]

            full = np.zeros((ROWS_PER_BLOCK, D), ml_dtypes.bfloat16)
            full[:rows] = xw_hi[r0:r0 + rows]
            xw_dev[bi] = full.reshape(TPB, P, D).transpose(1, 0, 2)

            fi = np.full(ROWS_PER_BLOCK, -1, np.int16)
            fi[:rows] = (labels_s[r0:r0 + rows] - s0).astype(np.int16) + \
                tile_base[:rows] + np.int16((bi % GB) * TPB * SEGS)
            idx_dev[bi] = fi.reshape(TPB, P)
            meta_c.append((int(s0), int(s1)))
        meta.append(meta_c)
        # idx layout on device: [P, B*TPB], column b*TPB+t
        in_maps.append({"xw": np.ascontiguousarray(
                            xw_dev.reshape(B, P, TPB * P)
                            .transpose(1, 0, 2).reshape(P, B * TPB * P)),
                        "idx": np.ascontiguousarray(
                            idx_dev.reshape(NT, P).T)})

    if B not in _COMPILED:
        _COMPILED[B] = _build_kernel(B)
    nc = _COMPILED[B]

    res = bass_utils.run_bass_kernel_spmd(nc, in_maps,
                                          core_ids=list(range(N_CORES)))

    # ---- gather / unshard
    out = np.zeros((NUM_SEGMENTS, D), np.float32)
    Z = 0.0
    for c in range(N_CORES):
        r = res.results[c]
        Z += float(r["zpart"].astype(np.float64).sum())
        out_dev = r["out"].reshape(SEGS, B, P).transpose(1, 0, 2)
        for bi in range(B):
            m = meta[c][bi]
            if m is None:
                continue
            s0, s1 = m
            out[s0:s1] = out_dev[bi, :s1 - s0, :].astype(np.float32)
    # pad rows have xw == 0 -> score 0 -> e = exp(0) = 1 each
    n_pad_rows = N_CORES * B * ROWS_PER_BLOCK - N
    Z -= float(n_pad_rows)
    out /= (w[None, :] * np.float32(Z))
    return out.astype(np.float32)


if __name__ == "__main__":
    from ref_io import get
    inputs, expected = get()
    out = kernel(**inputs)
    err = np.abs(out - expected)
    print("absmax err:", err.max(), "scale-rel:",
          err.max() / np.abs(expected).max())
